# revision 35
# baseline (speedup 1.0000x reference)
"""Trainium2 Bass kernel for GNN edge-softmax message passing.

Strategy (edge-parallel over 8 cores):
- Globally sort edges by dst, deal round-robin to cores (core c gets sorted
  edge 8i+c). Every core's tile t then covers the same global node window, so
  the per-tile node-block chunk lists (union over cores) are identical across
  cores -> a single SPMD program works.
- Per-edge hypernetwork matvecs (K, V) computed on DVE as broadcast-mult +
  grouped reduce over a host-packed layout [skw(32), skb, dkw(32), dkb] per
  output channel, with u_ext = [u, 1, v, 1].
- Softmax without segment-max (logits are O(1); exp never overflows; attn is
  mathematically identical). One AllReduce of a per-node accumulator
  acc[node, 36] = [sum exp (4 heads), sum V*exp (32)].
- Scatter-add and denominator gather via one-hot matmuls on the TensorEngine
  against 128-aligned node blocks (iota + tensor_scalar is_equal).
- in_feat[src]/in_feat[dst]/query[dst] gathers via gpsimd indirect DMA.
- Node-wise dynamic linear + relu + residual + LayerNorm on node shards
  (10 node blocks per core, selected via partition_id dynamic DMA offsets).
"""
import sys

sys.path.insert(0, "/opt/trn_rl_repo")

import numpy as np
import ml_dtypes

_BF16 = ml_dtypes.bfloat16
_NCORES = 8
_P = 128


class Cfg:
    def __init__(self, N=10000, E=50000, D=32, H=4, HD=8, ln_eps=1e-5):
        assert E % (_NCORES * _P) in range(_NCORES * _P)  # arbitrary E ok
        self.N, self.E, self.D, self.H, self.HD = N, E, D, H, HD
        self.ln_eps = ln_eps
        self.EPC = (E + _NCORES - 1) // _NCORES          # edges per core
        self.TE = (self.EPC + _P - 1) // _P              # edge tiles per core
        self.EPAD = self.TE * _P                         # padded edges/core
        real_blocks = (N + _P - 1) // _P                 # node blocks w/ data
        self.NBLK = ((real_blocks + 1 + _NCORES - 1) // _NCORES) * _NCORES
        self.REALBLK = real_blocks
        self.NROWS = self.NBLK * _P                      # padded node rows
        self.NTN = self.NBLK // _NCORES                  # node tiles per core
        self.SENT = real_blocks * _P                     # sentinel dst for pads
        self.ACCW = 4 + D                                # accumulator width


def _host_prepare(cfg, inputs):
    """Shard + repack all inputs on the host. Integer index work + pure data
    movement only (sorting, gathering, padding, layout)."""
    N, E, D, H, HD = cfg.N, cfg.E, cfg.D, cfg.H, cfg.HD
    NHD = H * HD
    f32 = np.float32
    src = np.asarray(inputs["src"]).astype(np.int32)
    dst = np.asarray(inputs["dst"]).astype(np.int32)
    in_feat = np.asarray(inputs["in_feat"], dtype=f32)
    query = np.asarray(inputs["query"], dtype=f32).reshape(N, NHD)
    node_w = np.asarray(inputs["node_w"], dtype=f32)
    node_b = np.asarray(inputs["node_b"], dtype=f32)
    ln_w = np.asarray(inputs["ln_w"], dtype=f32).reshape(1, D)
    ln_b = np.asarray(inputs["ln_b"], dtype=f32).reshape(1, D)

    order = np.argsort(dst, kind="stable")               # global sort by dst

    # padded gather tables (rows >= N are zero; sentinel row exists)
    ifp = np.zeros((cfg.NROWS, D), f32)
    ifp[:N] = in_feat
    qp = np.zeros((cfg.NROWS, NHD), f32)
    qp[:N] = query

    # pack edge weights: per output channel [skw, skb, dkw, dkb]
    def pack_w(wsrc, bsrc, wdst, bdst, idx):
        ws = np.asarray(wsrc, dtype=f32).reshape(E, NHD, D)[idx]
        wd = np.asarray(wdst, dtype=f32).reshape(E, NHD, D)[idx]
        bs = np.asarray(bsrc, dtype=f32).reshape(E, NHD, 1)[idx]
        bd = np.asarray(bdst, dtype=f32).reshape(E, NHD, 1)[idx]
        n = len(idx)
        out = np.zeros((cfg.EPAD, NHD * (2 * D + 2)), f32)
        out[:n] = np.concatenate([ws, bs, wd, bd], axis=2).reshape(n, -1)
        return out.astype(_BF16)

    in_maps, meta_perm = [], []
    for c in range(_NCORES):
        idx = order[c::_NCORES]                          # this core's edges
        n = len(idx)
        meta_perm.append(idx)
        dst_c = np.full(cfg.EPAD, cfg.SENT, np.int32)
        dst_c[:n] = dst[idx]
        src_c = np.zeros(cfg.EPAD, np.int32)
        src_c[:n] = src[idx]
        in_maps.append(dict(
            wk=pack_w(inputs["src_key_w"], inputs["src_key_b"],
                      inputs["dst_key_w"], inputs["dst_key_b"], idx),
            wv=pack_w(inputs["src_value_w"], inputs["src_value_b"],
                      inputs["dst_value_w"], inputs["dst_value_b"], idx),
            ifp=ifp, qp=qp,
            srcT=src_c.reshape(cfg.TE, _P).T.copy(),
            dstTi=dst_c.reshape(cfg.TE, _P).T.copy(),
            dst_c=dst_c,                                  # consumed below
        ))

    # union chunk lists per tile (identical across cores by construction)
    chunks = []
    for t in range(cfg.TE):
        blks = set()
        for m in in_maps:
            blks.update(np.unique(m["dst_c"][t * _P:(t + 1) * _P] // _P))
        blks = sorted(int(b) for b in blks)
        bb = blks[0]
        assert blks[-1] - bb < 8, f"tile {t} spans {blks}"
        chunks.append((bb, blks))

    for c, m in enumerate(in_maps):
        dst_c = m.pop("dst_c")
        dstloc = np.zeros((cfg.TE, _P), f32)
        for t in range(cfg.TE):
            dstloc[t] = dst_c[t * _P:(t + 1) * _P] - chunks[t][0] * _P
        assert dstloc.min() >= 0 and dstloc.max() < 1024
        m["dstloc"] = dstloc.T.copy()
        m["dstlocrow"] = dstloc.reshape(1, cfg.EPAD).copy()
        # node shard: blocks [c*NTN, (c+1)*NTN)
        lo, hi = c * cfg.NTN * _P, (c + 1) * cfg.NTN * _P
        nw_c = np.zeros((cfg.NTN * _P, D * (D + 1)), f32)
        rsd_c = np.zeros((cfg.NTN * _P, D), f32)
        lo_r = min(lo, N)
        hi_r = min(hi, N)
        if hi_r > lo_r:
            nwp = np.concatenate(
                [node_w[lo_r:hi_r], node_b[lo_r:hi_r, :, None]], axis=2)
            nw_c[:hi_r - lo_r] = nwp.reshape(hi_r - lo_r, -1)
            rsd_c[:hi_r - lo_r] = in_feat[lo_r:hi_r]
        m["nw"] = nw_c.astype(_BF16)
        m["rsd"] = rsd_c
        m["lnw"] = ln_w
        m["lnb"] = ln_b
    return in_maps, chunks, meta_perm


def _build_program(cfg, chunks, no_collective=False, no_indirect=False,
                   no_dynpid=False, te_limit=None, ntn_limit=None,
                   no_p2=False, no_p3a=False, no_p3b=False, p3b_stage=4):
    # p3b_stage=4 is the production path: tensor_tensor_reduce (stage 5)
    # crashes the device under this runtime, so variance uses TT+reduce.
    import concourse.bacc as bacc
    import concourse.bass as bass
    import concourse.mybir as mybir
    import concourse.tile as tile

    F = mybir.dt.float32
    BF = mybir.dt.bfloat16
    I32 = mybir.dt.int32
    Alu = mybir.AluOpType
    Act = mybir.ActivationFunctionType
    D, H, HD = cfg.D, cfg.H, cfg.HD
    NHD = H * HD
    WROW = NHD * (2 * D + 2)          # 2112: packed weight row
    UVW = 2 * D + 2                   # 66: u_ext width
    NWROW = D * (D + 1)               # 1056: packed node weight row
    ACCW = cfg.ACCW                   # 36

    nc = bacc.Bacc("TRN2", target_bir_lowering=False, debug=False,
                   num_devices=_NCORES)

    def din(name, shape, dt=F):
        return nc.dram_tensor(name, shape, dt, kind="ExternalInput")

    def dout(name, shape, dt=F):
        return nc.dram_tensor(name, shape, dt, kind="ExternalOutput")

    wk = din("wk", (cfg.EPAD, WROW), BF)
    wv = din("wv", (cfg.EPAD, WROW), BF)
    ifp = din("ifp", (cfg.NROWS, D))
    qp = din("qp", (cfg.NROWS, NHD))
    srcT = din("srcT", (_P, cfg.TE), I32)
    dstTi = din("dstTi", (_P, cfg.TE), I32)
    dstloc = din("dstloc", (_P, cfg.TE))
    dstlocrow = din("dstlocrow", (1, cfg.EPAD))
    nw = din("nw", (cfg.NTN * _P, NWROW), BF)
    rsd = din("rsd", (cfg.NTN * _P, D))
    lnw = din("lnw", (1, D))
    lnb = din("lnb", (1, D))
    key_s = dout("key_s", (cfg.EPAD, NHD))
    val_s = dout("val_s", (cfg.EPAD, NHD))
    attn_s = dout("attn_s", (cfg.EPAD, H))
    out_s = dout("out_s", (cfg.NTN * _P, D))

    with tile.TileContext(nc) as tc:
        with tc.tile_pool(name="persist", bufs=1) as pp, \
             tc.tile_pool(name="wpool", bufs=2) as wp, \
             tc.tile_pool(name="prodp", bufs=2) as prp, \
             tc.tile_pool(name="gath", bufs=3) as gp, \
             tc.tile_pool(name="small", bufs=4) as sp, \
             tc.tile_pool(name="ohp", bufs=4) as ohp, \
             tc.tile_pool(name="npool", bufs=2) as npo, \
             tc.tile_pool(name="pscat", bufs=2, space="PSUM") as pscat, \
             tc.tile_pool(name="pdn", bufs=2, space="PSUM") as pdn, \
             tc.tile_pool(name="pdbc", bufs=2, space="PSUM") as pdbc, \
             tc.tile_pool(name="pmisc", bufs=1, space="PSUM") as pmisc, \
             tc.tile_pool(name="dram", bufs=1, space="DRAM") as dp:

            # ---- persistent SBUF state ----
            acc = pp.tile([_P, cfg.NBLK * ACCW], F, tag="acc")
            nc.vector.memset(acc[:], 0.0)
            exall = pp.tile([_P, cfg.TE * H], F, tag="exall")
            iota1024 = pp.tile([_P, 1024], F, tag="iota1024")
            nc.gpsimd.iota(iota1024[:], pattern=[[1, 1024]], base=0,
                           channel_multiplier=0,
                           allow_small_or_imprecise_dtypes=True)
            iotablk = pp.tile([_P, 8], F, tag="iotablk")
            nc.gpsimd.iota(iotablk[:], pattern=[[_P, 8]], base=0,
                           channel_multiplier=1,
                           allow_small_or_imprecise_dtypes=True)
            srcT_sb = pp.tile([_P, cfg.TE], I32, tag="srcT")
            nc.sync.dma_start(srcT_sb[:], srcT[:])
            dstTi_sb = pp.tile([_P, cfg.TE], I32, tag="dstTi")
            nc.sync.dma_start(dstTi_sb[:], dstTi[:])
            dstloc_sb = pp.tile([_P, cfg.TE], F, tag="dstloc")
            nc.sync.dma_start(dstloc_sb[:], dstloc[:])
            dlr_sb = pp.tile([1, cfg.EPAD], F, tag="dlr")
            nc.sync.dma_start(dlr_sb[:], dstlocrow[:])
            ones_col = pp.tile([1, _P], F, tag="ones_col")
            nc.vector.memset(ones_col[:], 1.0)
            # ln_w/ln_b broadcast to all partitions via ones-matmul
            lnw_row = pp.tile([1, D], F, tag="lnw_row")
            nc.sync.dma_start(lnw_row[:], lnw[:])
            lnb_row = pp.tile([1, D], F, tag="lnb_row")
            nc.sync.dma_start(lnb_row[:], lnb[:])
            ln_ps = pmisc.tile([_P, 2 * D], F, space="PSUM", tag="lnps")
            nc.tensor.matmul(out=ln_ps[:, 0:D], lhsT=ones_col[:],
                             rhs=lnw_row[:], start=True, stop=True)
            nc.tensor.matmul(out=ln_ps[:, D:2 * D], lhsT=ones_col[:],
                             rhs=lnb_row[:], start=True, stop=True)
            lnw_b = pp.tile([_P, D], F, tag="lnw_b")
            nc.scalar.copy(lnw_b[:], ln_ps[:, 0:D])
            lnb_b = pp.tile([_P, D], F, tag="lnb_b")
            nc.scalar.copy(lnb_b[:], ln_ps[:, D:2 * D])

            # ---- phase 1: stream edge tiles ----
            TE_EFF = min(cfg.TE, te_limit) if te_limit else cfg.TE
            NTN_EFF = min(cfg.NTN, ntn_limit) if ntn_limit else cfg.NTN
            for t in range(TE_EFF):
                bb, blks = chunks[t]
                wk_t = wp.tile([_P, WROW], BF, tag="wk")
                nc.sync.dma_start(wk_t[:], wk[t * _P:(t + 1) * _P, :])
                wv_t = wp.tile([_P, WROW], BF, tag="wv")
                nc.sync.dma_start(wv_t[:], wv[t * _P:(t + 1) * _P, :])

                uvx = gp.tile([_P, UVW], F, tag="uvx")
                nc.vector.memset(uvx[:, D:D + 1], 1.0)
                nc.vector.memset(uvx[:, 2 * D + 1:2 * D + 2], 1.0)
                qg = gp.tile([_P, NHD], F, tag="qg")
                if no_indirect:
                    nc.vector.memset(uvx[:, 0:D], 0.01)
                    nc.vector.memset(uvx[:, D + 1:2 * D + 1], 0.01)
                    nc.vector.memset(qg[:], 0.01)
                else:
                    nc.gpsimd.indirect_dma_start(
                        out=uvx[:, 0:D], out_offset=None, in_=ifp[:],
                        in_offset=bass.IndirectOffsetOnAxis(
                            ap=srcT_sb[:, t:t + 1], axis=0))
                    nc.gpsimd.indirect_dma_start(
                        out=uvx[:, D + 1:2 * D + 1], out_offset=None,
                        in_=ifp[:],
                        in_offset=bass.IndirectOffsetOnAxis(
                            ap=dstTi_sb[:, t:t + 1], axis=0))
                    nc.gpsimd.indirect_dma_start(
                        out=qg[:], out_offset=None, in_=qp[:],
                        in_offset=bass.IndirectOffsetOnAxis(
                            ap=dstTi_sb[:, t:t + 1], axis=0))

                uvb = gp.tile([_P, UVW], BF, tag="uvb")
                nc.scalar.copy(uvb[:], uvx[:])
                uv_b = uvb[:].unsqueeze(1).to_broadcast([_P, NHD, UVW])
                # mults on gpsimd (Pool), grouped free-axis reduces on DVE
                # (gpsimd tensor_reduce only supports partition axes)
                prodk = prp.tile([_P, WROW], BF, tag="prodk")
                nc.gpsimd.tensor_tensor(
                    out=prodk[:].rearrange("p (c i) -> p c i", c=NHD),
                    in0=wk_t[:].rearrange("p (c i) -> p c i", c=NHD),
                    in1=uv_b, op=Alu.mult)
                k_t = sp.tile([_P, NHD], F, tag="k_t")
                nc.vector.reduce_sum(
                    out=k_t[:],
                    in_=prodk[:].rearrange("p (c i) -> p c i", c=NHD),
                    axis=mybir.AxisListType.X)
                prodv = prp.tile([_P, WROW], BF, tag="prodv")
                nc.gpsimd.tensor_tensor(
                    out=prodv[:].rearrange("p (c i) -> p c i", c=NHD),
                    in0=wv_t[:].rearrange("p (c i) -> p c i", c=NHD),
                    in1=uv_b, op=Alu.mult)
                v_t = sp.tile([_P, NHD], F, tag="v_t")
                nc.vector.reduce_sum(
                    out=v_t[:],
                    in_=prodv[:].rearrange("p (c i) -> p c i", c=NHD),
                    axis=mybir.AxisListType.X)

                nc.sync.dma_start(key_s[t * _P:(t + 1) * _P, :], k_t[:])
                nc.sync.dma_start(val_s[t * _P:(t + 1) * _P, :], v_t[:])

                # logits and exp
                kq = sp.tile([_P, NHD], F, tag="kq")
                nc.vector.tensor_tensor(out=kq[:], in0=k_t[:], in1=qg[:],
                                        op=Alu.mult)
                logit = sp.tile([_P, H], F, tag="logit")
                nc.vector.reduce_sum(
                    out=logit[:],
                    in_=kq[:].rearrange("p (h d) -> p h d", h=H),
                    axis=mybir.AxisListType.X)
                ex_sl = exall[:, t * H:(t + 1) * H]
                nc.scalar.activation(ex_sl, logit[:], Act.Exp)

                # payload [ex, V*ex]
                pay = sp.tile([_P, ACCW], F, tag="pay")
                nc.scalar.copy(pay[:, 0:H], ex_sl)
                ex_b = ex_sl.unsqueeze(2).to_broadcast([_P, H, HD])
                nc.vector.tensor_tensor(
                    out=pay[:, H:ACCW].rearrange("p (h d) -> p h d", h=H),
                    in0=v_t[:].rearrange("p (h d) -> p h d", h=H),
                    in1=ex_b, op=Alu.mult)

                # scatter into acc via one-hot matmuls per node block
                for blk in blks:
                    rel = blk - bb
                    en = ohp.tile([_P, _P], F, tag="en")
                    nc.vector.tensor_scalar(
                        out=en[:], in0=iota1024[:, rel * _P:(rel + 1) * _P],
                        scalar1=dstloc_sb[:, t:t + 1], scalar2=0.0,
                        op0=Alu.subtract, op1=Alu.is_equal)
                    psc = pscat.tile([_P, ACCW], F, space="PSUM", tag="psc")
                    nc.tensor.matmul(out=psc[:], lhsT=en[:], rhs=pay[:],
                                     start=True, stop=True)
                    a_sl = acc[:, blk * ACCW:(blk + 1) * ACCW]
                    nc.vector.tensor_add(out=a_sl, in0=a_sl, in1=psc[:])

            # ---- phase 2: AllReduce the accumulator ----
            ccin = dp.tile([_P, cfg.NBLK * ACCW], F, tag="ccin")
            ccout = nc.dram_tensor("ccout_sh", (_P, cfg.NBLK * ACCW), F,
                                   kind="Internal", addr_space="Shared")
            if not no_p2:
                nc.sync.dma_start(ccin[:], acc[:])
                if no_collective:
                    nc.sync.dma_start(ccout[:], ccin[:])
                else:
                    nc.gpsimd.collective_compute(
                        "AllReduce", Alu.add,
                        replica_groups=[list(range(_NCORES))],
                        ins=[ccin[:].opt()], outs=[ccout[:].opt()])
                nc.sync.dma_start(acc[:], ccout[:])

            # ---- phase 3a: attn = ex / denom[dst] ----
            for t in range(0 if no_p3a else TE_EFF):
                bb, blks = chunks[t]
                dbc = pdbc.tile([_P, _P], F, space="PSUM", tag="dbc")
                nc.tensor.matmul(out=dbc[:], lhsT=ones_col[:],
                                 rhs=dlr_sb[:, t * _P:(t + 1) * _P],
                                 start=True, stop=True)
                dnp = pdn.tile([_P, H], F, space="PSUM", tag="dnp")
                for ci, blk in enumerate(blks):
                    rel = blk - bb
                    ne = ohp.tile([_P, _P], F, tag="ne")
                    nc.vector.tensor_scalar(
                        out=ne[:], in0=dbc[:],
                        scalar1=iotablk[:, rel:rel + 1], scalar2=0.0,
                        op0=Alu.subtract, op1=Alu.is_equal)
                    nc.tensor.matmul(
                        out=dnp[:], lhsT=ne[:],
                        rhs=acc[:, blk * ACCW:blk * ACCW + H],
                        start=(ci == 0), stop=(ci == len(blks) - 1))
                dnr = sp.tile([_P, H], F, tag="dnr")
                nc.vector.reciprocal(dnr[:], dnp[:])
                at_t = sp.tile([_P, H], F, tag="at_t")
                nc.vector.tensor_tensor(out=at_t[:],
                                        in0=exall[:, t * H:(t + 1) * H],
                                        in1=dnr[:], op=Alu.mult)
                nc.sync.dma_start(attn_s[t * _P:(t + 1) * _P, :], at_t[:])

            # ---- phase 3b: node-wise linear + relu + residual + LN ----
            pid = nc.partition_id()
            for j in range(0 if no_p3b else NTN_EFF):
                agg_t = npo.tile([_P, ACCW], F, tag="agg_t")
                if no_dynpid:
                    nc.sync.dma_start(agg_t[:],
                                      ccout[:, bass.ds(j * ACCW, ACCW)])
                else:
                    nc.sync.dma_start(
                        agg_t[:],
                        ccout[:, bass.ds(pid * (cfg.NTN * ACCW) + j * ACCW,
                                         ACCW)])
                if p3b_stage <= 1:
                    y1 = npo.tile([_P, D], F, tag="y1")
                    nc.scalar.copy(y1[:], agg_t[:, H:ACCW])
                    nc.sync.dma_start(out_s[j * _P:(j + 1) * _P, :], y1[:])
                    continue
                dn = npo.tile([_P, H], F, tag="dn")
                nc.vector.tensor_scalar(out=dn[:], in0=agg_t[:, 0:H],
                                        scalar1=1e-30, scalar2=None,
                                        op0=Alu.add)
                dvr = npo.tile([_P, H], F, tag="dvr")
                nc.vector.reciprocal(dvr[:], dn[:])
                agx = npo.tile([_P, D + 1], F, tag="agx")
                nc.vector.memset(agx[:, D:D + 1], 1.0)
                dvr_b = dvr[:].unsqueeze(2).to_broadcast([_P, H, HD])
                nc.vector.tensor_tensor(
                    out=agx[:, 0:D].rearrange("p (h d) -> p h d", h=H),
                    in0=agg_t[:, H:ACCW].rearrange("p (h d) -> p h d", h=H),
                    in1=dvr_b, op=Alu.mult)
                nw_t = npo.tile([_P, NWROW], BF, tag="nw_t")
                nc.sync.dma_start(nw_t[:], nw[j * _P:(j + 1) * _P, :])
                agxb = npo.tile([_P, D + 1], BF, tag="agxb")
                nc.scalar.copy(agxb[:], agx[:])
                prodn = npo.tile([_P, NWROW], BF, tag="prodn")
                agx_b = agxb[:].unsqueeze(1).to_broadcast([_P, D, D + 1])
                nc.gpsimd.tensor_tensor(
                    out=prodn[:].rearrange("p (o i) -> p o i", o=D),
                    in0=nw_t[:].rearrange("p (o i) -> p o i", o=D),
                    in1=agx_b, op=Alu.mult)
                lin = npo.tile([_P, D], F, tag="lin")
                nc.vector.reduce_sum(
                    out=lin[:],
                    in_=prodn[:].rearrange("p (o i) -> p o i", o=D),
                    axis=mybir.AxisListType.X)
                if p3b_stage <= 2:
                    nc.sync.dma_start(out_s[j * _P:(j + 1) * _P, :], lin[:])
                    continue
                nc.scalar.activation(lin[:], lin[:], Act.Relu)
                rsd_t = npo.tile([_P, D], F, tag="rsd_t")
                nc.sync.dma_start(rsd_t[:], rsd[j * _P:(j + 1) * _P, :])
                x = npo.tile([_P, D], F, tag="x")
                nc.vector.tensor_add(out=x[:], in0=lin[:], in1=rsd_t[:])
                if p3b_stage <= 3:
                    nc.sync.dma_start(out_s[j * _P:(j + 1) * _P, :], x[:])
                    continue
                # layernorm
                mu = npo.tile([_P, 1], F, tag="mu")
                nc.vector.reduce_sum(out=mu[:], in_=x[:],
                                     axis=mybir.AxisListType.X)
                nc.scalar.mul(mu[:], mu[:], 1.0 / D)
                xc = npo.tile([_P, D], F, tag="xc")
                nc.vector.tensor_scalar(out=xc[:], in0=x[:],
                                        scalar1=mu[:, :1], scalar2=None,
                                        op0=Alu.subtract)
                scrap = npo.tile([_P, D], F, tag="scrap")
                var = npo.tile([_P, 1], F, tag="var")
                if p3b_stage <= 4:
                    nc.vector.tensor_tensor(out=scrap[:], in0=xc[:],
                                            in1=xc[:], op=Alu.mult)
                    nc.vector.reduce_sum(out=var[:], in_=scrap[:],
                                         axis=mybir.AxisListType.X)
                    nc.vector.tensor_scalar(out=var[:], in0=var[:],
                                            scalar1=1.0 / D, scalar2=None,
                                            op0=Alu.mult)
                else:
                    nc.vector.tensor_tensor_reduce(
                        out=scrap[:], in0=xc[:], in1=xc[:], scale=1.0 / D,
                        scalar=0.0, op0=Alu.mult, op1=Alu.add,
                        accum_out=var[:])
                sd = npo.tile([_P, 1], F, tag="sd")
                nc.vector.tensor_scalar(out=sd[:], in0=var[:],
                                        scalar1=cfg.ln_eps, scalar2=None,
                                        op0=Alu.add)
                nc.scalar.sqrt(sd[:], sd[:])
                rstd = npo.tile([_P, 1], F, tag="rstd")
                nc.vector.reciprocal(rstd[:], sd[:])
                y = npo.tile([_P, D], F, tag="y")
                nc.vector.tensor_scalar(out=y[:], in0=xc[:],
                                        scalar1=rstd[:, :1], scalar2=None,
                                        op0=Alu.mult)
                nc.vector.tensor_tensor(out=y[:], in0=y[:], in1=lnw_b[:],
                                        op=Alu.mult)
                nc.vector.tensor_tensor(out=y[:], in0=y[:], in1=lnb_b[:],
                                        op=Alu.add)
                nc.sync.dma_start(out_s[j * _P:(j + 1) * _P, :], y[:])

    nc.compile()
    return nc


def _assemble(cfg, results, meta_perm):
    N, E, D, H = cfg.N, cfg.E, cfg.D, cfg.H
    out = np.zeros((N, D), np.float32)
    key_feat = np.zeros((E, D), np.float32)
    value_feat = np.zeros((E, D), np.float32)
    attn = np.zeros((E, H), np.float32)
    for c in range(_NCORES):
        r = results[c]
        idx = meta_perm[c]
        n = len(idx)
        key_feat[idx] = r["key_s"][:n]
        value_feat[idx] = r["val_s"][:n]
        attn[idx] = r["attn_s"][:n]
        lo = c * cfg.NTN * _P
        hi = min((c + 1) * cfg.NTN * _P, N)
        if hi > lo:
            out[lo:hi] = r["out_s"][:hi - lo]
    return out, key_feat, value_feat, attn


_PROGRAM_CACHE = {}


def make_runner(nc):
    """Build a reusable jitted SPMD callable (mirrors bass2jax's multi-core
    path) so repeated executions skip retracing/recompiling."""
    import jax
    import numpy as _np
    from jax.sharding import Mesh, PartitionSpec
    from jax.experimental.shard_map import shard_map
    import concourse.mybir as mybir
    from concourse import bass2jax
    bass2jax.install_neuronx_cc_hook()
    partition_name = (nc.partition_id_tensor.name
                      if nc.partition_id_tensor else None)
    in_names, out_names, out_avals, zero_outs = [], [], [], []
    for alloc in nc.m.functions[0].allocations:
        if not isinstance(alloc, mybir.MemoryLocationSet):
            continue
        name = alloc.memorylocations[0].name
        if alloc.kind == "ExternalInput":
            if name != partition_name:
                in_names.append(name)
        elif alloc.kind == "ExternalOutput":
            shape = tuple(alloc.tensor_shape)
            dtype = mybir.dt.np(alloc.dtype)
            out_names.append(name)
            out_avals.append(jax.core.ShapedArray(shape, dtype))
            zero_outs.append(_np.zeros(shape, dtype))
    n_params = len(in_names)
    all_in_names = list(in_names) + list(out_names)
    if partition_name is not None:
        all_in_names.append(partition_name)

    def _body(*args):
        operands = list(args)
        if partition_name is not None:
            operands.append(bass2jax.partition_id_tensor())
        return tuple(bass2jax._bass_exec_p.bind(
            *operands, out_avals=tuple(out_avals),
            in_names=tuple(all_in_names), out_names=tuple(out_names),
            lowering_input_output_aliases=(),
            sim_require_finite=True, sim_require_nnan=True, nc=nc))

    devices = jax.devices()[:_NCORES]
    mesh = Mesh(_np.asarray(devices), ("core",))
    n_outs = len(out_names)
    sharded = jax.jit(
        shard_map(_body, mesh=mesh,
                  in_specs=(PartitionSpec("core"),) * (n_params + n_outs),
                  out_specs=(PartitionSpec("core"),) * n_outs,
                  check_rep=False),
        keep_unused=True)
    return sharded, in_names, out_names, out_avals, zero_outs, mesh


def run_repeated(inputs, cfg=None, iters=5):
    """Correctness + timing: returns (outputs, list of per-iter seconds)."""
    import jax, time
    import numpy as _np
    cfg = cfg or Cfg()
    in_maps, chunks, meta_perm = _host_prepare(cfg, inputs)
    key = (cfg.N, cfg.E, tuple(tuple(b for b in bl) for _, bl in chunks))
    if key not in _PROGRAM_CACHE:
        _PROGRAM_CACHE[key] = _build_program(cfg, chunks)
    nc = _PROGRAM_CACHE[key]
    from jax.sharding import NamedSharding, PartitionSpec as _PS
    sharded, in_names, out_names, out_avals, zero_outs, mesh = make_runner(nc)
    sh = NamedSharding(mesh, _PS("core"))
    concat_in = [_np.concatenate([_np.asarray(in_maps[c][n])
                                  for c in range(_NCORES)], axis=0)
                 for n in in_names]
    concat_zeros = [_np.zeros((_NCORES * z.shape[0], *z.shape[1:]), z.dtype)
                    for z in zero_outs]
    # pre-shard so the timed calls involve zero host<->device movement
    dev_in = [jax.device_put(a, sh) for a in concat_in]
    dev_zero = [jax.device_put(a, sh) for a in concat_zeros]
    jax.block_until_ready(dev_in + dev_zero)
    out_arrs = sharded(*dev_in, *dev_zero)
    jax.block_until_ready(out_arrs)
    results = [{name: _np.asarray(out_arrs[i]).reshape(
                    _NCORES, *out_avals[i].shape)[c]
                for i, name in enumerate(out_names)}
               for c in range(_NCORES)]
    times = []
    for _ in range(iters):
        t0 = time.perf_counter()
        o = sharded(*dev_in, *dev_zero)
        jax.block_until_ready(o)
        times.append(time.perf_counter() - t0)
    # pipelined batch: amortizes per-call dispatch latency over the tunnel
    t0 = time.perf_counter()
    outs = [sharded(*dev_in, *dev_zero) for _ in range(iters)]
    jax.block_until_ready(outs)
    pipelined = (time.perf_counter() - t0) / iters
    times.append(pipelined)
    return _assemble(cfg, results, meta_perm), times


def run(inputs, cfg=None, **spmd_kwargs):
    from concourse import bass_utils
    cfg = cfg or Cfg()
    in_maps, chunks, meta_perm = _host_prepare(cfg, inputs)
    key = (cfg.N, cfg.E, tuple(tuple(b for b in bl) for _, bl in chunks))
    if key not in _PROGRAM_CACHE:
        _PROGRAM_CACHE[key] = _build_program(cfg, chunks)
    nc = _PROGRAM_CACHE[key]
    res = bass_utils.run_bass_kernel_spmd(
        nc, in_maps, core_ids=list(range(_NCORES)), **spmd_kwargs)
    return _assemble(cfg, res.results, meta_perm), res


def kernel(**inputs):
    (out, key_feat, value_feat, attn), _ = run(inputs)
    return out, key_feat, value_feat, attn
